# revision 2
# baseline (speedup 1.0000x reference)
"""CliffordLinear kernel for Trainium2 (8 NeuronCores, data parallel).

The reference applies 2016 sequential Givens rotations (one per (i,j) pair,
i<j, dim=64) to every row of x, then adds a bias. Each rotation is linear in
x, so the whole sequence composes into a single 64x64 matrix R with
out = x @ R + bias. R is computed on the host (float64, 2016 tiny updates);
the device does a streaming matmul.

The device pass is HBM-bandwidth-bound (~358 GB/s per NeuronCore), so both
streams are fp16: x is cast+packed on host, the matmul runs fp16*fp16 with
fp32 PSUM accumulation, and the output is stored as fp16 and upcast on host.
Quantization error is ~5e-4 relative — far under the 2e-2 gate — and fp16
halves the DMA bytes vs fp32 (and runs the PE at full rate instead of 1/4).

Device layout: the tensor engine contracts over the partition axis, so x is
pre-arranged on host into tiles of [128, TILE_COLS] where partition
p = b*64+d holds feature d of row-block b (two 32768-row blocks stacked).
The stationary weight is W = blockdiag(R, R) [128, 128] so one matmul
processes both blocks with all 128 partitions active. Tiles are stored
tile-major in DRAM ([T, 128, C]) so every DMA is one contiguous 1 MiB block.

Per tile, 8 matmuls of 512 columns accumulate into PSUM; the PSUM->SBUF
drain (fused with the bias add) alternates between the Vector and Scalar
engines — a single engine reading PSUM at 1 elem/cycle would rival the DMA
roofline. Loads ride the SP HWDGE ring, stores the ACT ring, so the two
rings split the traffic and neither queue head-of-line blocks the other.
"""

import numpy as np

DIM = 64
NROWS = 524288
NCORES = 8
SHARD = NROWS // NCORES  # 65536 rows per core
HALF = SHARD // 2        # 32768 columns per stacked block
TILE_COLS = 4096         # columns per DMA tile (128*4096*2 = 1 MiB fp16)
MM_COLS = 512            # moving-operand columns per matmul (one PSUM bank)

_BASS_CACHE = {}


def _compose_rotation(coeffs64):
    """R such that applying the reference rotation sequence == x @ R."""
    ii, jj = np.triu_indices(DIM, k=1)
    c = np.cos(coeffs64)
    s = np.sin(coeffs64)
    R = np.eye(DIM, dtype=np.float64)
    for k in range(len(ii)):
        i, j = int(ii[k]), int(jj[k])
        ri = R[:, i].copy()
        rj = R[:, j].copy()
        R[:, i] = c[k] * ri - s[k] * rj
        R[:, j] = s[k] * ri + c[k] * rj
    return R


def _pack_shard(xs, tile_cols, np_dt=np.float16):
    """(SHARD, DIM) -> [T, 128, tile_cols] tile-major device layout."""
    t = HALF // tile_cols
    x2 = xs.astype(np_dt).reshape(2, HALF, DIM).transpose(0, 2, 1).reshape(
        128, HALF)
    return np.ascontiguousarray(
        x2.reshape(128, t, tile_cols).transpose(1, 0, 2)
    )


def _unpack_shard(o3, tile_cols):
    """[T, 128, tile_cols] -> (SHARD, DIM) fp32."""
    o2 = np.asarray(o3).transpose(1, 0, 2).reshape(128, HALF)
    return o2.reshape(2, DIM, HALF).transpose(0, 2, 1).reshape(
        SHARD, DIM).astype(np.float32)


def _build_bass(half=HALF, tile_cols=TILE_COLS, n_cores=NCORES, reps=1,
                mm_dtype="f16", mode="split", io_bufs=3, ps_bufs=8):
    import concourse.bass as bass
    import concourse.bacc as bacc
    import concourse.mybir as mybir
    import concourse.tile as tile

    f32 = mybir.dt.float32
    fmm = {"f32": f32, "f32r": mybir.dt.float32r,
           "bf16": mybir.dt.bfloat16, "f16": mybir.dt.float16}[mm_dtype]
    fout = fmm if mm_dtype in ("bf16", "f16") else f32
    nc = bacc.Bacc(
        "TRN2", target_bir_lowering=False, debug=False, num_devices=n_cores
    )
    n_tiles = half // tile_cols
    mm_per_tile = tile_cols // MM_COLS

    x_d = nc.dram_tensor("x2", [n_tiles, 128, tile_cols], fmm,
                         kind="ExternalInput")
    w_d = nc.dram_tensor("w", [128, 128], fmm, kind="ExternalInput")
    b_d = nc.dram_tensor("b2", [128, 1], f32, kind="ExternalInput")
    o_d = nc.dram_tensor("o2", [n_tiles, 128, tile_cols], fout,
                         kind="ExternalOutput")

    ident = mybir.ActivationFunctionType.Identity

    with tile.TileContext(nc) as tc:
        with (
            tc.tile_pool(name="const", bufs=1) as cpool,
            tc.tile_pool(name="io", bufs=io_bufs) as iopool,
            tc.tile_pool(name="xp", bufs=1) as xpool,
            tc.tile_pool(name="ps", bufs=ps_bufs,
                         space=bass.MemorySpace.PSUM) as pspool,
        ):
            w = cpool.tile([128, 128], fmm)
            nc.sync.dma_start(w[:], w_d[:])
            bb = cpool.tile([128, 1], f32)
            nc.sync.dma_start(bb[:], b_d[:])
            for _rep in range(reps):
                xins = []
                for t in range(n_tiles):
                    xin = xpool.tile([128, tile_cols], fmm, tag=f"xin{t}")
                    nc.sync.dma_start(xin[:], x_d[t])
                    xins.append(xin)
                for t in range(n_tiles):
                    out = iopool.tile([128, tile_cols], fout, tag="out")
                    for u in range(mm_per_tile):
                        ps = pspool.tile([128, MM_COLS], f32)
                        nc.tensor.matmul(
                            ps[:],
                            w[:],
                            xins[t][:, u * MM_COLS:(u + 1) * MM_COLS],
                            start=True,
                            stop=True,
                        )
                        oc = out[:, u * MM_COLS:(u + 1) * MM_COLS]
                        if u % 2 == 0:
                            nc.vector.tensor_scalar_add(oc, ps[:], bb[:])
                        else:
                            nc.scalar.activation(oc, ps[:], ident,
                                                 bias=bb[:], scale=1.0)
                    nc.scalar.dma_start(o_d[t], out[:])
    nc.compile()
    return nc


def kernel(x, bivector_coeffs, bias):
    from concourse.bass_utils import run_bass_kernel_spmd

    x = np.ascontiguousarray(np.asarray(x, dtype=np.float32))
    coeffs = np.asarray(bivector_coeffs, dtype=np.float64)
    bias = np.asarray(bias, dtype=np.float32)

    R = _compose_rotation(coeffs)
    W = np.zeros((128, 128), dtype=np.float16)
    W[:DIM, :DIM] = R.astype(np.float16)
    W[DIM:, DIM:] = R.astype(np.float16)
    b2 = np.ascontiguousarray(np.tile(bias, 2).reshape(128, 1))

    key = (HALF, TILE_COLS, NCORES, "f16", "split")
    if key not in _BASS_CACHE:
        _BASS_CACHE[key] = _build_bass(
            half=HALF, tile_cols=TILE_COLS, n_cores=NCORES, reps=1,
            mm_dtype="f16", mode="split",
        )
    nc = _BASS_CACHE[key]

    in_maps = []
    for r in range(NCORES):
        xs = x[r * SHARD:(r + 1) * SHARD]
        in_maps.append(
            {"x2": _pack_shard(xs, TILE_COLS), "w": W, "b2": b2}
        )

    res = run_bass_kernel_spmd(
        nc, in_maps, core_ids=list(range(NCORES)), trace=False
    )

    out = np.empty((NROWS, DIM), dtype=np.float32)
    for r in range(NCORES):
        out[r * SHARD:(r + 1) * SHARD] = _unpack_shard(
            res.results[r]["o2"], TILE_COLS
        )
    return out


# revision 8
# speedup vs baseline: 9.5063x; 9.5063x over previous
"""CliffordLinear kernel for Trainium2 (8 NeuronCores, data parallel).

The reference applies 2016 sequential Givens rotations (one per (i,j) pair,
i<j, dim=64) to every row of x, then adds a bias. Each rotation is linear in
x, so the whole sequence composes into a single 64x64 matrix R with
out = x @ R + bias. R is computed on the host (float64, 2016 tiny updates);
the device does a streaming matmul.

The device pass is HBM-bandwidth-bound (~358 GB/s per NeuronCore), so both
streams are fp16: x is cast+packed on host, the matmul runs fp16*fp16 with
fp32 PSUM accumulation, and the output is stored as fp16 and upcast on host.
Quantization error is ~5e-4 relative — far under the 2e-2 gate — and fp16
halves the DMA bytes vs fp32 (and runs the PE at full rate instead of 1/4).

Device layout: the tensor engine contracts over the partition axis, so x is
pre-arranged on host into tiles of [128, TILE_COLS] where partition
p = b*64+d holds feature d of row-block b (two 32768-row blocks stacked).
The stationary weight is W = blockdiag(R, R) [128, 128] so one matmul
processes both blocks with all 128 partitions active. Tiles are stored
tile-major in DRAM ([T, 128, C]) so every DMA is one contiguous 1 MiB block.

Per tile, 8 matmuls of 512 columns accumulate into PSUM; the PSUM->SBUF
drain (fused with the bias add) alternates between the Vector and Scalar
engines — a single engine reading PSUM at 1 elem/cycle would rival the DMA
roofline. Loads ride the SP HWDGE ring, stores the ACT ring, so the two
rings split the traffic and neither queue head-of-line blocks the other.
"""

import numpy as np

DIM = 64
NROWS = 524288
NCORES = 8
SHARD = NROWS // NCORES  # 65536 rows per core
HALF = SHARD // 2        # 32768 columns per stacked block
TILE_COLS = 4096         # columns per DMA tile (128*4096*2 = 1 MiB fp16)
MM_COLS = 512            # moving-operand columns per matmul (one PSUM bank)

_BASS_CACHE = {}


def _compose_rotation(coeffs64):
    """R such that applying the reference rotation sequence == x @ R."""
    ii, jj = np.triu_indices(DIM, k=1)
    c = np.cos(coeffs64)
    s = np.sin(coeffs64)
    R = np.eye(DIM, dtype=np.float64)
    for k in range(len(ii)):
        i, j = int(ii[k]), int(jj[k])
        ri = R[:, i].copy()
        rj = R[:, j].copy()
        R[:, i] = c[k] * ri - s[k] * rj
        R[:, j] = s[k] * ri + c[k] * rj
    return R


def _pack_shard(xs, tile_cols, np_dt=np.float16):
    """(SHARD, DIM) -> [T, 128, tile_cols] tile-major device layout."""
    t = HALF // tile_cols
    x2 = xs.astype(np_dt).reshape(2, HALF, DIM).transpose(0, 2, 1).reshape(
        128, HALF)
    return np.ascontiguousarray(
        x2.reshape(128, t, tile_cols).transpose(1, 0, 2)
    )


def _unpack_shard(o3, tile_cols):
    """[T, 128, tile_cols] -> (SHARD, DIM) fp32."""
    o2 = np.asarray(o3).transpose(1, 0, 2).reshape(128, HALF)
    return o2.reshape(2, DIM, HALF).transpose(0, 2, 1).reshape(
        SHARD, DIM).astype(np.float32)


def _build_bass(half=HALF, tile_cols=TILE_COLS, n_cores=NCORES, reps=1,
                mm_dtype="f16", mode="split", io_bufs=3, ps_bufs=8,
                mm_cols=MM_COLS, ring="dedic", tiny_out=False, x_bufs=1):
    import concourse.bass as bass
    import concourse.bacc as bacc
    import concourse.mybir as mybir
    import concourse.tile as tile

    f32 = mybir.dt.float32
    fmm = {"f32": f32, "f32r": mybir.dt.float32r,
           "bf16": mybir.dt.bfloat16, "f16": mybir.dt.float16}[mm_dtype]
    fout = fmm if mm_dtype in ("bf16", "f16") else f32
    nc = bacc.Bacc(
        "TRN2", target_bir_lowering=False, debug=False, num_devices=n_cores
    )
    n_tiles = half // tile_cols
    mm_per_tile = tile_cols // mm_cols
    ps_cols = min(mm_cols, 512 if mm_dtype in ("f32", "f32r") else 1024)
    assert mm_cols == ps_cols

    x_d = nc.dram_tensor("x2", [n_tiles, 128, tile_cols], fmm,
                         kind="ExternalInput")
    w_d = nc.dram_tensor("w", [128, 128], fmm, kind="ExternalInput")
    b_d = nc.dram_tensor("b2", [128, 1], f32, kind="ExternalInput")
    # tiny_out: timing builds keep every DMA/compute identical but land o2 in
    # an Internal DRAM scratch tensor, exposing only a 512 B real output —
    # returning the full 16 MiB per call through the axon tunnel costs an
    # unstable 10-80 ms that swamps the per-rep timing signal.
    o_d = nc.dram_tensor("o2", [n_tiles, 128, tile_cols], fout,
                         kind="Internal" if tiny_out else "ExternalOutput")
    s_d = (nc.dram_tensor("osmall", [128, 1], f32, kind="ExternalOutput")
           if tiny_out else None)

    ident = mybir.ActivationFunctionType.Identity

    with tile.TileContext(nc) as tc:
        with (
            tc.tile_pool(name="const", bufs=1) as cpool,
            tc.tile_pool(name="io", bufs=io_bufs) as iopool,
            tc.tile_pool(name="xp", bufs=x_bufs) as xpool,
            tc.tile_pool(name="ps", bufs=ps_bufs,
                         space=bass.MemorySpace.PSUM) as pspool,
        ):
            w = cpool.tile([128, 128], fmm)
            nc.sync.dma_start(w[:], w_d[:])
            bb = cpool.tile([128, 1], f32)
            nc.sync.dma_start(bb[:], b_d[:])
            for _rep in range(reps):
                xins = []
                for t in range(n_tiles):
                    xin = xpool.tile([128, tile_cols], fmm, tag=f"xin{t}")
                    ld = nc.sync if (ring == "dedic" or t % 2 == 0) \
                        else nc.scalar
                    ld.dma_start(xin[:], x_d[t])
                    xins.append(xin)
                for t in range(n_tiles):
                    out = iopool.tile([128, tile_cols], fout, tag="out")
                    for u in range(mm_per_tile):
                        ps = pspool.tile([128, mm_cols], f32)
                        nc.tensor.matmul(
                            ps[:],
                            w[:],
                            xins[t][:, u * mm_cols:(u + 1) * mm_cols],
                            start=True,
                            stop=True,
                        )
                        oc = out[:, u * mm_cols:(u + 1) * mm_cols]
                        if u % 2 == 0:
                            nc.vector.tensor_scalar_add(oc, ps[:], bb[:])
                        else:
                            nc.scalar.activation(oc, ps[:], ident,
                                                 bias=bb[:], scale=1.0)
                    st = nc.scalar if (ring == "dedic" or t % 2 == 1) \
                        else nc.sync
                    st.dma_start(o_d[t], out[:])
            if s_d is not None:
                nc.sync.dma_start(s_d[:], bb[:])
    nc.compile()
    return nc


def kernel(x, bivector_coeffs, bias):
    from concourse.bass_utils import run_bass_kernel_spmd

    x = np.ascontiguousarray(np.asarray(x, dtype=np.float32))
    coeffs = np.asarray(bivector_coeffs, dtype=np.float64)
    bias = np.asarray(bias, dtype=np.float32)

    R = _compose_rotation(coeffs)
    W = np.zeros((128, 128), dtype=np.float16)
    W[:DIM, :DIM] = R.astype(np.float16)
    W[DIM:, DIM:] = R.astype(np.float16)
    b2 = np.ascontiguousarray(np.tile(bias, 2).reshape(128, 1))

    key = (HALF, TILE_COLS, NCORES, "f16", "split")
    if key not in _BASS_CACHE:
        _BASS_CACHE[key] = _build_bass(
            half=HALF, tile_cols=TILE_COLS, n_cores=NCORES, reps=1,
            mm_dtype="f16", mode="split",
        )
    nc = _BASS_CACHE[key]

    in_maps = []
    for r in range(NCORES):
        xs = x[r * SHARD:(r + 1) * SHARD]
        in_maps.append(
            {"x2": _pack_shard(xs, TILE_COLS), "w": W, "b2": b2}
        )

    res = run_bass_kernel_spmd(
        nc, in_maps, core_ids=list(range(NCORES)), trace=False
    )

    out = np.empty((NROWS, DIM), dtype=np.float32)
    for r in range(NCORES):
        out[r * SHARD:(r + 1) * SHARD] = _unpack_shard(
            res.results[r]["o2"], TILE_COLS
        )
    return out
